# revision 18
# baseline (speedup 1.0000x reference)
"""Trainium2 Bass kernel for the NSDE model (Euler-Maruyama scan + MLPs).

Strategy (v3):
  - Data-parallel over batch: 16384 rows -> 8 cores x 2048 rows.
  - Only the 20 time slices of x_path the scan reads are shipped (host
    gathers indices from t_span).
  - Feature-major packed layout: activations are [feature x 2-half, 512]
    so every matmul runs with K=128 (blockdiag / row-tiled weights) and
    elementwise ops use all 128 partitions.
  - Two interleaved streams of 1024 rows hide the serial scan latency.
  - ALL matmuls are bf16 (f32r streams at half rate and its weight loads
    cost 2x; removing it halves PE time). The h carry stays exact: a
    plain fp32 SBUF tensor updated only by the vector engine
    (h' = (ps3 + dt*db3) + h), never fed to the PE.
  - The noise term sigmoid*zs enters the drift-out PSUM via a bf16
    identity matmul, so the h update is a single scalar_tensor_tensor.
  - A bf16 copy of h (for the drift/diffusion matmul inputs) is made
    each step with a fast 2x-mode vector copy.
  - Drift l1/l2 PSUM is one 2-bank [128,1024] tile per layer so each
    relu is a single wide activation pass (fewer per-op overheads).
  - Step 0 is specialized for h=0: diffusion is a host-computed constant
    folded into the step-0 noise; drift l1 is x-only.
  - No PE warm-up burst: the first steps themselves un-gate the HAM
    clock; a dedicated burst costs more than the cold penalty it saves.
"""

import os
from contextlib import ExitStack

import ml_dtypes
import numpy as np

import concourse.bass as bass
import concourse.mybir as mybir
import concourse.tile as tile
from concourse import bacc
from concourse.bass_utils import run_bass_kernel_spmd

F32 = mybir.dt.float32
BF16 = mybir.dt.bfloat16
AF = mybir.ActivationFunctionType
ALU = mybir.AluOpType

NPBF = ml_dtypes.bfloat16

STEPS = 20
NCORES = 8
B = 16384
BC = B // NCORES  # per-core batch: 2048
SB = BC // 2  # per-stream batch: 1024
HB = SB // 2  # packed free size per stream: 512
H = 64
FX = 64
DW = 128

# cbh (bf16, hot: step-0 critical) column offsets
OF_W1X = 0
OF_W2 = 128
OF_IDN = 256
OF_W30 = 384
CBH_COLS = 448
# cbr (bf16, rest) column offsets
OF_W3R = 0  # steps 1..19
OF_W1H = 19 * H
OF_GW1 = OF_W1H + 128
OF_GW2 = OF_GW1 + 128
OF_RW1 = OF_GW2 + 128
OF_RW2 = OF_RW1 + 64
CBR_COLS = OF_RW2 + 4

_CACHE = {}


def _build():
    if "nc" in _CACHE:
        return _CACHE["nc"]

    nc = bacc.Bacc("TRN2", target_bir_lowering=False, debug=False)

    def din(name, shape, dt=F32):
        return nc.dram_tensor(name, shape, dt, kind="ExternalInput")

    d_xt = din("xt", [STEPS, 128, 2, HB], BF16)  # [step, feat-packed, stream, b]
    d_zst = din("zst", [STEPS, 128, 2, HB], BF16)
    d_wu = din("wu", [128, 640], BF16)  # PE warm-up fodder (tiny, DMA'd first)
    d_cbh = din("cbh", [128, CBH_COLS], BF16)
    d_cbr = din("cbr", [128, CBR_COLS], BF16)
    # f32: b1|b2|cdtb3(20)|gb1|gb2|rb1bd|rb2q -> 26 cols
    d_cf = din("cf", [128, 26])
    d_out = nc.dram_tensor("out", [4, 2, HB], F32, kind="ExternalOutput")

    with ExitStack() as ctx:
        tc = ctx.enter_context(tile.TileContext(nc))
        consts = ctx.enter_context(tc.tile_pool(name="consts", bufs=1))
        xzp = ctx.enter_context(tc.tile_pool(name="xzp", bufs=8))
        hbp = ctx.enter_context(tc.tile_pool(name="hbp", bufs=3))
        wk = ctx.enter_context(tc.tile_pool(name="wk", bufs=3))
        # 8 PSUM banks: ph 2 (persistent h carry) + ppb 2x2 (wide l1/l2) + pps 2
        ph = ctx.enter_context(tc.tile_pool(name="ph", bufs=1, space="PSUM"))
        ppb = ctx.enter_context(tc.tile_pool(name="ppb", bufs=2, space="PSUM"))
        pps = ctx.enter_context(tc.tile_pool(name="pps", bufs=2, space="PSUM"))

        def cload(dram_ap, shape, name, dt=F32):
            t = consts.tile(shape, dt, name=name, tag=name)
            nc.sync.dma_start(t[:], dram_ap)
            return t

        wu = cload(d_wu[:, :], [128, 640], "wu", BF16)
        cbh = cload(d_cbh[:, :], [128, CBH_COLS], "cbh", BF16)
        cf = cload(d_cf[:, :], [128, 26], "cf")

        # PE warm-up: ~4.3us of dense cold matmuls flips the HAM clock gate
        # to 8/8 before step 0; reads only the tiny first DMA.
        psw = pps.tile([128, HB], F32, name="psw", tag="pps")
        for _ in range(10):
            nc.tensor.matmul(
                psw[:, :], wu[:, 0:128], wu[:, 128:640],
                start=True, stop=True, skip_group_check=True,
            )

        cbr = cload(d_cbr[:, :], [128, CBR_COLS], "cbr", BF16)
        w1h = cbr[:, OF_W1H : OF_W1H + 128]
        w1x = cbh[:, OF_W1X : OF_W1X + 128]
        w2 = cbh[:, OF_W2 : OF_W2 + 128]
        gw1 = cbr[:, OF_GW1 : OF_GW1 + 128]
        gw2 = cbr[:, OF_GW2 : OF_GW2 + 128]
        idn = cbh[:, OF_IDN : OF_IDN + 128]
        rw1 = cbr[:, OF_RW1 : OF_RW1 + 64]
        rw2 = cbr[0:64, OF_RW2 : OF_RW2 + 4]
        b1 = cf[:, 0:1]
        b2 = cf[:, 1:2]
        cdtb3 = cf[:, 2:22]  # cumulative sum of dt*db3 per step
        gb1 = cf[:, 22:23]
        gb2 = cf[:, 23:24]
        rb1 = cf[0:64, 24:25]
        rb2 = cf[0:4, 25:26]

        def w3k(k):
            if k == 0:
                return cbh[:, OF_W30 : OF_W30 + H]
            return cbr[:, OF_W3R + H * (k - 1) : OF_W3R + H * k]

        def dma_xz(k):
            xkb = xzp.tile([128, 2, HB], BF16, name="xkb", tag="xkb")
            nc.sync.dma_start(xkb[:], d_xt[k])
            zkb = xzp.tile([128, 2, HB], BF16, name="zkb", tag="zkb")
            nc.sync.dma_start(zkb[:], d_zst[k])
            return xkb, zkb

        # prefetch first few steps' x/z
        xz = {k: dma_xz(k) for k in range(3)}

        # persistent PSUM h carry (one bank per stream); matmuls accumulate
        # drift*dt and the noise term into it, fp32-exact across all steps
        hps = [ph.tile([128, HB], F32, name=f"hps{s}", tag=f"hps{s}") for s in (0, 1)]
        hb_cur = [None, None]  # bf16 copy (h + cum dt*db3) for matmul inputs

        def mm(out, lhsT, rhs, **kw):
            nc.tensor.matmul(out, lhsT, rhs, skip_group_check=True, **kw)

        def e_ps1_h(s, ps1):
            hb = hb_cur[s]
            mm(ps1[:, 0:HB], w1h[0:64, :], hb[0:64, :], start=True, stop=False)
            mm(ps1[:, HB:], w1h[64:128, :], hb[64:128, :],
               start=True, stop=False, tile_position=(64, 0))

        def e_ps1_x(s, xk, ps1, acc):
            mm(ps1[:, 0:HB], w1x[0:64, :], xk[0:64, :], start=not acc, stop=True)
            mm(ps1[:, HB:], w1x[64:128, :], xk[64:128, :],
               start=not acc, stop=True, tile_position=(64, 0))

        def e_ps2(z1, ps2):
            mm(ps2[:, 0:HB], w2[:, :], z1[:, 0:HB], start=True, stop=True)
            mm(ps2[:, HB:], w2[:, :], z1[:, HB:], start=True, stop=True)

        def e_l3(s, k, z2, first=False):
            mm(hps[s][0:64, :], w3k(k), z2[:, 0:HB], start=False, stop=False)
            mm(hps[s][64:128, :], w3k(k), z2[:, HB:],
               start=False, stop=False, tile_position=(0, 64))

        def e_ident(s, noise, first=False):
            mm(hps[s][:, :], idn[:, :], noise, start=first, stop=False)

        def e_hbcast(s, k):
            hb = hbp.tile([128, HB], BF16, name=f"hb{s}", tag=f"hb{s}")
            nc.scalar.activation(
                hb[:], hps[s][:, :], AF.Identity, bias=cdtb3[:, k : k + 1]
            )
            hb_cur[s] = hb

        # ---- step 0: h = 0 (x-only drift; diffusion folded into zst[0]) ----
        xkb, zkb = xz.pop(0)
        ps1w, z1t, z2t = {}, {}, {}
        for s in range(2):
            e_ident(s, zkb[:, s, :], first=True)
        for s in range(2):
            ps1w[s] = ppb.tile([128, SB], F32, name=f"ps1{s}", tag="ppb")
            e_ps1_x(s, xkb[:, s, :], ps1w[s], acc=False)
        for s in range(2):
            z1t[s] = wk.tile([128, SB], BF16, name=f"z1{s}", tag=f"z1{s}")
            nc.scalar.activation(z1t[s][:], ps1w[s][:], AF.Relu, bias=b1[:])
        for s in range(2):
            ps2 = ppb.tile([128, SB], F32, name=f"ps2{s}", tag="ppb")
            e_ps2(z1t[s], ps2)
            z2t[s] = wk.tile([128, SB], BF16, name=f"z2{s}", tag=f"z2{s}")
            nc.vector.tensor_scalar(z2t[s][:], ps2[:], b2[:], 0.0, ALU.add, ALU.max)
            e_l3(s, 0, z2t[s])
        for s in range(2):
            e_hbcast(s, 0)

        # ---- steps 1..19: hand-ordered emission so each engine's FIFO
        # receives ops in expected input-ready order (streams phase-offset) ----
        def step_pair(k):
            if k + 2 < STEPS and (k + 2) not in xz:
                xz[k + 2] = dma_xz(k + 2)
            xkb, zkb = xz.pop(k)
            psg, g1, pss, sg, tt, ps1, ps2, z1, z2 = ({} for _ in range(9))

            def w(pool, shape, nm, s, dt=BF16):
                return pool.tile(shape, dt, name=f"{nm}{s}", tag=f"{nm}{s}")

            for s in range(2):
                # --- PE: diffusion l1 + drift l1 for stream s ---
                psg[s] = pps.tile([128, HB], F32, name=f"psg{s}", tag="pps")
                nc.tensor.matmul(
                    psg[s][:, :], gw1[:, :], hb_cur[s][:, :], start=True, stop=True
                )
                ps1[s] = ppb.tile([128, SB], F32, name=f"ps1{s}", tag="ppb")
                e_ps1_h(s, ps1[s])
                e_ps1_x(s, xkb[:, s, :], ps1[s], acc=True)
                # --- DVE: g1 relu (fast, feeds pss) ---
                g1[s] = w(wk, [128, HB], "g1", s)
                nc.vector.tensor_scalar(
                    g1[s][:], psg[s][:], gb1[:], 0.0, ALU.add, ALU.max
                )
                # --- PE: diffusion l2 ---
                pss[s] = pps.tile([128, HB], F32, name=f"pss{s}", tag="pps")
                nc.tensor.matmul(
                    pss[s][:, :], gw2[:, :], g1[s][:, :], start=True, stop=True
                )
                # --- ACT: drift l1 relu (wide) then sigmoid ---
                z1[s] = w(wk, [128, SB], "z1", s)
                nc.scalar.activation(z1[s][:], ps1[s][:], AF.Relu, bias=b1[:])
                sg[s] = w(wk, [128, HB], "sg", s)
                nc.scalar.activation(sg[s][:], pss[s][:], AF.Sigmoid, bias=gb2[:])
                # --- GPSIMD (idle engine): noise mul, parallel to DVE ---
                tt[s] = w(wk, [128, HB], "tt", s)
                nc.gpsimd.tensor_mul(tt[s][:], sg[s][:], zkb[:, s, :])
                # --- PE: drift l2 ---
                ps2[s] = ppb.tile([128, SB], F32, name=f"ps2{s}", tag="ppb")
                e_ps2(z1[s], ps2[s])
                # --- PE: noise into h psum ---
                e_ident(s, tt[s][:, :])
                # --- DVE: drift l2 relu (wide) ---
                z2[s] = w(wk, [128, SB], "z2", s)
                nc.vector.tensor_scalar(
                    z2[s][:], ps2[s][:], b2[:], 0.0, ALU.add, ALU.max
                )
                # --- PE: drift l3 into h psum ---
                e_l3(s, k, z2[s])
                # --- ACT: h+cum-bias -> bf16 for next step ---
                e_hbcast(s, k)

        for k in range(1, STEPS):
            step_pair(k)

        # ---- readout: out = relu(h @ rW1 + rb1) @ rW2 + rb2 ----
        osb = wk.tile([4, 2, HB], F32, name="osb", tag="osb")
        for s in range(2):
            psr = pps.tile([128, HB], F32, name="psr", tag="pps")
            nc.tensor.matmul(
                psr[0:64, :], rw1[:, :], hb_cur[s][:, :], start=True, stop=True
            )
            r1 = wk.tile([64, HB], BF16, name=f"r1{s}", tag=f"r1{s}")
            nc.scalar.activation(r1[:], psr[0:64, :], AF.Relu, bias=rb1[:])
            pso = pps.tile([128, HB], F32, name="pso", tag="pps")
            nc.tensor.matmul(
                pso[0:4, :], rw2[:, :], r1[:, :], start=True, stop=True
            )
            nc.scalar.activation(
                osb[:, s, :], pso[0:4, :], AF.Identity, bias=rb2[:]
            )
        nc.sync.dma_start(d_out[:, :, :], osb[:])

    nc.compile()
    _CACHE["nc"] = nc
    return nc


def _dup(a, dt=NPBF):
    return np.ascontiguousarray(np.concatenate([a, a], axis=0).astype(dt))


def _blkdiag(a, dt=NPBF):
    n, m = a.shape
    out = np.zeros((2 * n, 2 * m), np.float32)
    out[:n, :m] = a
    out[n:, m:] = a
    return np.ascontiguousarray(out.astype(dt))


def _sigmoid(x):
    return 1.0 / (1.0 + np.exp(-x))


def _prep_in_maps(inputs):
    xp = np.asarray(inputs["x_path"], dtype=np.float32)
    t_span = np.asarray(inputs["t_span"], dtype=np.float32)
    dw = np.asarray(inputs["dW"], dtype=np.float32)

    Tm1 = np.int32(xp.shape[1] - 1)
    t_max = t_span[-1]
    idx = np.clip(
        (t_span[:-1] / t_max * np.float32(Tm1)).astype(np.int32), 0, Tm1
    )
    dts = (t_span[1:] - t_span[:-1]).astype(np.float32)
    sq = np.sqrt(dts).astype(np.float32)

    gscale = np.asarray(inputs["gscale"], dtype=np.float32)
    w1 = np.asarray(inputs["dW1"], dtype=np.float32)
    w2 = np.asarray(inputs["dW2"], dtype=np.float32)
    w3 = np.asarray(inputs["dW3"], dtype=np.float32)
    db1 = np.asarray(inputs["db1"], dtype=np.float32)
    db2 = np.asarray(inputs["db2"], dtype=np.float32)
    db3 = np.asarray(inputs["db3"], dtype=np.float32)
    gw1 = np.asarray(inputs["gW1"], dtype=np.float32)
    gw2 = np.asarray(inputs["gW2"], dtype=np.float32)
    gb1 = np.asarray(inputs["gb1"], dtype=np.float32)
    gb2 = np.asarray(inputs["gb2"], dtype=np.float32)
    rw1 = np.asarray(inputs["rW1"], dtype=np.float32)
    rb1 = np.asarray(inputs["rb1"], dtype=np.float32)
    rw2 = np.asarray(inputs["rW2"], dtype=np.float32)
    rb2 = np.asarray(inputs["rb2"], dtype=np.float32)

    w3s = w3[None, :, :] * dts[:, None, None]  # [STEPS, DW, H]
    w3s_flat = w3s.transpose(1, 0, 2).reshape(DW, STEPS * H)

    def pad128(a):
        out = np.zeros((128, a.shape[1]), a.dtype)
        out[: a.shape[0]] = a
        return out

    cbh_pack = np.concatenate(
        [
            _dup(w1[H:]),  # w1x
            w2.astype(NPBF),  # w2
            np.eye(DW, dtype=np.float32).astype(NPBF),  # ident
            w3s_flat[:, 0:H].astype(NPBF),  # w3s step 0
        ],
        axis=1,
    )
    cbr_pack = np.concatenate(
        [
            w3s_flat[:, H:].astype(NPBF),  # w3s steps 1..19
            _dup(w1[:H]),  # w1h
            _blkdiag(gw1),  # gw1
            _blkdiag(gw2),  # gw2
            _blkdiag(rw1),  # rw1 [128, 64]
            pad128(_blkdiag(rw2)),  # rw2 [128, 4]
        ],
        axis=1,
    )
    cdtb3 = np.cumsum(dts[:, None] * db3[None, :], axis=0)  # [STEPS, H]
    cf_pack = np.concatenate(
        [
            db1.reshape(DW, 1),
            db2.reshape(DW, 1),
            _dup(cdtb3.T, np.float32),
            _dup(gb1.reshape(H, 1), np.float32),
            _dup(gb2.reshape(H, 1), np.float32),
            pad128(_dup(rb1.reshape(32, 1), np.float32)),
            pad128(_dup(rb2.reshape(2, 1), np.float32)),
        ],
        axis=1,
    ).astype(np.float32)

    common = {
        "wu": np.full((128, 640), 0.5, NPBF),
        "cbh": np.ascontiguousarray(cbh_pack),
        "cbr": np.ascontiguousarray(cbr_pack),
        "cf": np.ascontiguousarray(cf_pack),
    }

    xg = xp[:, idx, :]  # [B, STEPS, F]
    zsc = (gscale[None, :] * sq[:, None]).copy()  # [STEPS, H]
    # step 0: h=0 so the sigmoid factor is the constant sg0; fold into zs
    sg0 = _sigmoid(np.maximum(gb1, 0.0) @ gw2 + gb2)  # [H]
    zsc[0] *= sg0

    in_maps = []
    for c in range(NCORES):
        rows = slice(c * BC, (c + 1) * BC)
        # (stream, half, b', k, f) -> (k, half, f, stream, b')
        xt = np.ascontiguousarray(
            xg[rows]
            .reshape(2, 2, HB, STEPS, FX)
            .transpose(3, 1, 4, 0, 2)
            .reshape(STEPS, 128, 2, HB)
            .astype(NPBF)
        )
        zc = dw[:, rows, :] * zsc[:, None, :]  # [STEPS, BC, H]
        zst = np.ascontiguousarray(
            zc.reshape(STEPS, 2, 2, HB, H)
            .transpose(0, 2, 4, 1, 3)
            .reshape(STEPS, 128, 2, HB)
            .astype(NPBF)
        )
        m = dict(common)
        m["xt"] = xt
        m["zst"] = zst
        in_maps.append(m)
    return in_maps


def kernel(**inputs):
    nc = _build()
    in_maps = _prep_in_maps(inputs)
    run_kwargs = dict(_CACHE.get("run_kwargs", {}))
    res = run_bass_kernel_spmd(nc, in_maps, list(range(NCORES)), **run_kwargs)
    _CACHE["last_results"] = res
    mus, lss = [], []
    for c in range(NCORES):
        o = res.results[c]["out"]  # [(mu_h0,ls_h0,mu_h1,ls_h1), stream, b]
        mus.append(np.concatenate([o[0, 0], o[2, 0], o[0, 1], o[2, 1]]))
        lss.append(np.concatenate([o[1, 0], o[3, 0], o[1, 1], o[3, 1]]))
    mu = np.concatenate(mus)
    ls = np.concatenate(lss)
    return mu, ls


# revision 23
# speedup vs baseline: 1.1474x; 1.1474x over previous
"""Trainium2 Bass kernel for the NSDE model (Euler-Maruyama scan + MLPs).

Strategy (v3):
  - Data-parallel over batch: 16384 rows -> 8 cores x 2048 rows.
  - Only the 20 time slices of x_path the scan reads are shipped (host
    gathers indices from t_span).
  - Feature-major packed layout: activations are [feature x 2-half, 512]
    so every matmul runs with K=128 (blockdiag / row-tiled weights) and
    elementwise ops use all 128 partitions.
  - Two interleaved streams of 1024 rows hide the serial scan latency.
  - ALL matmuls are bf16 (f32r streams at half rate and its weight loads
    cost 2x; removing it halves PE time). The h carry stays exact: a
    plain fp32 SBUF tensor updated only by the vector engine
    (h' = (ps3 + dt*db3) + h), never fed to the PE.
  - The noise term sigmoid*zs enters the drift-out PSUM via a bf16
    identity matmul, so the h update is a single scalar_tensor_tensor.
  - A bf16 copy of h (for the drift/diffusion matmul inputs) is made
    each step with a fast 2x-mode vector copy.
  - Drift l1/l2 PSUM is one 2-bank [128,1024] tile per layer so each
    relu is a single wide activation pass (fewer per-op overheads).
  - Step 0 is specialized for h=0: diffusion is a host-computed constant
    folded into the step-0 noise; drift l1 is x-only.
  - No PE warm-up burst: the first steps themselves un-gate the HAM
    clock; a dedicated burst costs more than the cold penalty it saves.
"""

import os
from contextlib import ExitStack

import ml_dtypes
import numpy as np

import concourse.bass as bass
import concourse.mybir as mybir
import concourse.tile as tile
from concourse import bacc
from concourse.bass_utils import run_bass_kernel_spmd

F32 = mybir.dt.float32
BF16 = mybir.dt.bfloat16
AF = mybir.ActivationFunctionType
ALU = mybir.AluOpType

NPBF = ml_dtypes.bfloat16

STEPS = 20
NCORES = 8
B = 16384
BC = B // NCORES  # per-core batch: 2048
SB = BC // 2  # per-stream batch: 1024
HB = SB // 2  # packed free size per stream: 512
H = 64
FX = 64
DW = 128

# cbh (bf16, hot: step-0 critical) column offsets
OF_W1X = 0
OF_W2 = 128
OF_IDN = 256
OF_W30 = 384
CBH_COLS = 448
# cbr (bf16, rest) column offsets
OF_W3R = 0  # steps 1..19
OF_W1H = 19 * H
OF_GW1 = OF_W1H + 128
OF_GW2 = OF_GW1 + 128
OF_RW1 = OF_GW2 + 128
OF_RW2 = OF_RW1 + 64
CBR_COLS = OF_RW2 + 4

_CACHE = {}


def _build():
    if "nc" in _CACHE:
        return _CACHE["nc"]

    nc = bacc.Bacc("TRN2", target_bir_lowering=False, debug=False)

    def din(name, shape, dt=F32):
        return nc.dram_tensor(name, shape, dt, kind="ExternalInput")

    d_xt = din("xt", [STEPS, 128, 2, HB], BF16)  # [step, feat-packed, stream, b]
    d_zst = din("zst", [STEPS, 128, 2, HB], BF16)
    d_cbh = din("cbh", [128, CBH_COLS], BF16)
    d_cbr = din("cbr", [128, CBR_COLS], BF16)
    # f32: b1|b2|cdtb3(20)|gb1|gb2|rb1bd|rb2q -> 26 cols
    d_cf = din("cf", [128, 26])
    d_out = nc.dram_tensor("out", [4, 2, HB], F32, kind="ExternalOutput")

    with ExitStack() as ctx:
        tc = ctx.enter_context(tile.TileContext(nc))
        consts = ctx.enter_context(tc.tile_pool(name="consts", bufs=1))
        xzp = ctx.enter_context(tc.tile_pool(name="xzp", bufs=8))
        hbp = ctx.enter_context(tc.tile_pool(name="hbp", bufs=3))
        wk = ctx.enter_context(tc.tile_pool(name="wk", bufs=3))
        # 8 PSUM banks: ph 2 (persistent h carry) + ppb 2x2 (wide l1/l2) + pps 2
        ph = ctx.enter_context(tc.tile_pool(name="ph", bufs=1, space="PSUM"))
        ppb = ctx.enter_context(tc.tile_pool(name="ppb", bufs=2, space="PSUM"))
        pps = ctx.enter_context(tc.tile_pool(name="pps", bufs=2, space="PSUM"))

        def cload(dram_ap, shape, name, dt=F32):
            t = consts.tile(shape, dt, name=name, tag=name)
            nc.sync.dma_start(t[:], dram_ap)
            return t

        # step-0 inputs first: each DMA issue occupies the sync queue ~600ns,
        # so order by first use
        xkb0 = xzp.tile([128, 2, HB], BF16, name="xkb", tag="xkb")
        nc.sync.dma_start(xkb0[:], d_xt[0])
        zkb0 = xzp.tile([128, 2, HB], BF16, name="zkb", tag="zkb")
        nc.sync.dma_start(zkb0[:], d_zst[0])
        cbh = cload(d_cbh[:, :], [128, CBH_COLS], "cbh", BF16)
        cf = cload(d_cf[:, :], [128, 26], "cf")
        cbr = cload(d_cbr[:, :], [128, CBR_COLS], "cbr", BF16)
        w1h = cbr[:, OF_W1H : OF_W1H + 128]
        w1x = cbh[:, OF_W1X : OF_W1X + 128]
        w2 = cbh[:, OF_W2 : OF_W2 + 128]
        gw1 = cbr[:, OF_GW1 : OF_GW1 + 128]
        gw2 = cbr[:, OF_GW2 : OF_GW2 + 128]
        idn = cbh[:, OF_IDN : OF_IDN + 128]
        rw1 = cbr[:, OF_RW1 : OF_RW1 + 64]
        rw2 = cbr[0:64, OF_RW2 : OF_RW2 + 4]
        b1 = cf[:, 0:1]
        b2 = cf[:, 1:2]
        cdtb3 = cf[:, 2:22]  # cumulative sum of dt*db3 per step
        gb1 = cf[:, 22:23]
        gb2 = cf[:, 23:24]
        rb1 = cf[0:64, 24:25]
        rb2 = cf[0:4, 25:26]

        def w3k(k):
            if k == 0:
                return cbh[:, OF_W30 : OF_W30 + H]
            return cbr[:, OF_W3R + H * (k - 1) : OF_W3R + H * k]

        def dma_xz(k):
            xkb = xzp.tile([128, 2, HB], BF16, name="xkb", tag="xkb")
            nc.sync.dma_start(xkb[:], d_xt[k])
            zkb = xzp.tile([128, 2, HB], BF16, name="zkb", tag="zkb")
            nc.sync.dma_start(zkb[:], d_zst[k])
            return xkb, zkb

        # prefetch first few steps' x/z (step 0 already issued above)
        xz = {0: (xkb0, zkb0), 1: dma_xz(1), 2: dma_xz(2)}

        # persistent PSUM h carry (one bank per stream); matmuls accumulate
        # drift*dt and the noise term into it, fp32-exact across all steps
        hps = [ph.tile([128, HB], F32, name=f"hps{s}", tag=f"hps{s}") for s in (0, 1)]
        hb_cur = [None, None]  # bf16 copy (h + cum dt*db3) for matmul inputs

        def mm(out, lhsT, rhs, **kw):
            nc.tensor.matmul(out, lhsT, rhs, skip_group_check=True, **kw)

        def e_ps1_h(s, ps1):
            hb = hb_cur[s]
            mm(ps1[:, 0:HB], w1h[0:64, :], hb[0:64, :], start=True, stop=False)
            mm(ps1[:, HB:], w1h[64:128, :], hb[64:128, :],
               start=True, stop=False, tile_position=(64, 0))

        def e_ps1_x(s, xk, ps1, acc):
            mm(ps1[:, 0:HB], w1x[0:64, :], xk[0:64, :], start=not acc, stop=True)
            mm(ps1[:, HB:], w1x[64:128, :], xk[64:128, :],
               start=not acc, stop=True, tile_position=(64, 0))

        def e_ps2(z1, ps2):
            mm(ps2[:, 0:HB], w2[:, :], z1[:, 0:HB], start=True, stop=True)
            mm(ps2[:, HB:], w2[:, :], z1[:, HB:], start=True, stop=True)

        def e_l3(s, k, z2, first=False):
            mm(hps[s][0:64, :], w3k(k), z2[:, 0:HB], start=False, stop=False)
            mm(hps[s][64:128, :], w3k(k), z2[:, HB:],
               start=False, stop=False, tile_position=(0, 64))

        def e_ident(s, noise, first=False):
            mm(hps[s][:, :], idn[:, :], noise, start=first, stop=False)

        def e_hbcast(s, k):
            hb = hbp.tile([128, HB], BF16, name=f"hb{s}", tag=f"hb{s}")
            nc.scalar.activation(
                hb[:], hps[s][:, :], AF.Identity, bias=cdtb3[:, k : k + 1]
            )
            hb_cur[s] = hb

        # ---- step 0: h = 0 (x-only drift; diffusion folded into zst[0]) ----
        xkb, zkb = xz.pop(0)
        ps1w, z1t, z2t = {}, {}, {}
        for s in range(2):
            e_ident(s, zkb[:, s, :], first=True)
        for s in range(2):
            ps1w[s] = ppb.tile([128, SB], F32, name=f"ps1{s}", tag="ppb")
            e_ps1_x(s, xkb[:, s, :], ps1w[s], acc=False)
        for s in range(2):
            z1t[s] = wk.tile([128, SB], BF16, name=f"z1{s}", tag=f"z1{s}")
            nc.scalar.activation(z1t[s][:], ps1w[s][:], AF.Relu, bias=b1[:])
        for s in range(2):
            ps2 = ppb.tile([128, SB], F32, name=f"ps2{s}", tag="ppb")
            e_ps2(z1t[s], ps2)
            z2t[s] = wk.tile([128, SB], BF16, name=f"z2{s}", tag=f"z2{s}")
            nc.vector.tensor_scalar(z2t[s][:], ps2[:], b2[:], 0.0, ALU.add, ALU.max)
            e_l3(s, 0, z2t[s])
        for s in range(2):
            e_hbcast(s, 0)

        # ---- steps 1..19: hand-ordered emission so each engine's FIFO
        # receives ops in expected input-ready order (streams phase-offset) ----
        def step_pair(k):
            if k + 2 < STEPS and (k + 2) not in xz:
                xz[k + 2] = dma_xz(k + 2)
            xkb, zkb = xz.pop(k)
            psg, g1, pss, sg, tt, ps1, ps2, z1, z2 = ({} for _ in range(9))

            def w(pool, shape, nm, s, dt=BF16):
                return pool.tile(shape, dt, name=f"{nm}{s}", tag=f"{nm}{s}")

            for s in range(2):
                # --- PE: diffusion l1 + drift l1 for stream s ---
                psg[s] = pps.tile([128, HB], F32, name=f"psg{s}", tag="pps")
                nc.tensor.matmul(
                    psg[s][:, :], gw1[:, :], hb_cur[s][:, :], start=True, stop=True
                )
                ps1[s] = ppb.tile([128, SB], F32, name=f"ps1{s}", tag="ppb")
                e_ps1_h(s, ps1[s])
                e_ps1_x(s, xkb[:, s, :], ps1[s], acc=True)
                # --- DVE: g1 relu (fast, feeds pss) ---
                g1[s] = w(wk, [128, HB], "g1", s)
                nc.vector.tensor_scalar(
                    g1[s][:], psg[s][:], gb1[:], 0.0, ALU.add, ALU.max
                )
                # --- PE: diffusion l2 ---
                pss[s] = pps.tile([128, HB], F32, name=f"pss{s}", tag="pps")
                nc.tensor.matmul(
                    pss[s][:, :], gw2[:, :], g1[s][:, :], start=True, stop=True
                )
                # --- DVE: drift l1 relu (wide, early in step) ---
                z1[s] = w(wk, [128, SB], "z1", s)
                nc.vector.tensor_scalar(
                    z1[s][:], ps1[s][:], b1[:], 0.0, ALU.add, ALU.max
                )
                # --- ACT: sigmoid ---
                sg[s] = w(wk, [128, HB], "sg", s)
                nc.scalar.activation(sg[s][:], pss[s][:], AF.Sigmoid, bias=gb2[:])
                # --- DVE: noise mul (bf16 2x, slots into DVE gap after z1) ---
                tt[s] = w(wk, [128, HB], "tt", s)
                nc.vector.tensor_tensor(tt[s][:], sg[s][:], zkb[:, s, :], ALU.mult)
                # --- PE: drift l2 ---
                ps2[s] = ppb.tile([128, SB], F32, name=f"ps2{s}", tag="ppb")
                e_ps2(z1[s], ps2[s])
                # --- PE: noise into h psum ---
                e_ident(s, tt[s][:, :])
                # --- ACT: drift l2 relu (wide, late in step) ---
                z2[s] = w(wk, [128, SB], "z2", s)
                nc.scalar.activation(z2[s][:], ps2[s][:], AF.Relu, bias=b2[:])
                # --- PE: drift l3 into h psum ---
                e_l3(s, k, z2[s])
                # --- ACT: h+cum-bias -> bf16 for next step ---
                e_hbcast(s, k)

        for k in range(1, STEPS):
            step_pair(k)

        # ---- readout: out = relu(h @ rW1 + rb1) @ rW2 + rb2 ----
        osb = wk.tile([4, 2, HB], F32, name="osb", tag="osb")
        for s in range(2):
            psr = pps.tile([128, HB], F32, name="psr", tag="pps")
            nc.tensor.matmul(
                psr[0:64, :], rw1[:, :], hb_cur[s][:, :], start=True, stop=True
            )
            r1 = wk.tile([64, HB], BF16, name=f"r1{s}", tag=f"r1{s}")
            nc.scalar.activation(r1[:], psr[0:64, :], AF.Relu, bias=rb1[:])
            pso = pps.tile([128, HB], F32, name="pso", tag="pps")
            nc.tensor.matmul(
                pso[0:4, :], rw2[:, :], r1[:, :], start=True, stop=True
            )
            nc.scalar.activation(
                osb[:, s, :], pso[0:4, :], AF.Identity, bias=rb2[:]
            )
        nc.sync.dma_start(d_out[:, :, :], osb[:])

    nc.compile()
    _CACHE["nc"] = nc
    return nc


def _dup(a, dt=NPBF):
    return np.ascontiguousarray(np.concatenate([a, a], axis=0).astype(dt))


def _blkdiag(a, dt=NPBF):
    n, m = a.shape
    out = np.zeros((2 * n, 2 * m), np.float32)
    out[:n, :m] = a
    out[n:, m:] = a
    return np.ascontiguousarray(out.astype(dt))


def _sigmoid(x):
    return 1.0 / (1.0 + np.exp(-x))


def _prep_in_maps(inputs):
    xp = np.asarray(inputs["x_path"], dtype=np.float32)
    t_span = np.asarray(inputs["t_span"], dtype=np.float32)
    dw = np.asarray(inputs["dW"], dtype=np.float32)

    Tm1 = np.int32(xp.shape[1] - 1)
    t_max = t_span[-1]
    idx = np.clip(
        (t_span[:-1] / t_max * np.float32(Tm1)).astype(np.int32), 0, Tm1
    )
    dts = (t_span[1:] - t_span[:-1]).astype(np.float32)
    sq = np.sqrt(dts).astype(np.float32)

    gscale = np.asarray(inputs["gscale"], dtype=np.float32)
    w1 = np.asarray(inputs["dW1"], dtype=np.float32)
    w2 = np.asarray(inputs["dW2"], dtype=np.float32)
    w3 = np.asarray(inputs["dW3"], dtype=np.float32)
    db1 = np.asarray(inputs["db1"], dtype=np.float32)
    db2 = np.asarray(inputs["db2"], dtype=np.float32)
    db3 = np.asarray(inputs["db3"], dtype=np.float32)
    gw1 = np.asarray(inputs["gW1"], dtype=np.float32)
    gw2 = np.asarray(inputs["gW2"], dtype=np.float32)
    gb1 = np.asarray(inputs["gb1"], dtype=np.float32)
    gb2 = np.asarray(inputs["gb2"], dtype=np.float32)
    rw1 = np.asarray(inputs["rW1"], dtype=np.float32)
    rb1 = np.asarray(inputs["rb1"], dtype=np.float32)
    rw2 = np.asarray(inputs["rW2"], dtype=np.float32)
    rb2 = np.asarray(inputs["rb2"], dtype=np.float32)

    w3s = w3[None, :, :] * dts[:, None, None]  # [STEPS, DW, H]
    w3s_flat = w3s.transpose(1, 0, 2).reshape(DW, STEPS * H)

    def pad128(a):
        out = np.zeros((128, a.shape[1]), a.dtype)
        out[: a.shape[0]] = a
        return out

    cbh_pack = np.concatenate(
        [
            _dup(w1[H:]),  # w1x
            w2.astype(NPBF),  # w2
            np.eye(DW, dtype=np.float32).astype(NPBF),  # ident
            w3s_flat[:, 0:H].astype(NPBF),  # w3s step 0
        ],
        axis=1,
    )
    cbr_pack = np.concatenate(
        [
            w3s_flat[:, H:].astype(NPBF),  # w3s steps 1..19
            _dup(w1[:H]),  # w1h
            _blkdiag(gw1),  # gw1
            _blkdiag(gw2),  # gw2
            _blkdiag(rw1),  # rw1 [128, 64]
            pad128(_blkdiag(rw2)),  # rw2 [128, 4]
        ],
        axis=1,
    )
    cdtb3 = np.cumsum(dts[:, None] * db3[None, :], axis=0)  # [STEPS, H]
    cf_pack = np.concatenate(
        [
            db1.reshape(DW, 1),
            db2.reshape(DW, 1),
            _dup(cdtb3.T, np.float32),
            _dup(gb1.reshape(H, 1), np.float32),
            _dup(gb2.reshape(H, 1), np.float32),
            pad128(_dup(rb1.reshape(32, 1), np.float32)),
            pad128(_dup(rb2.reshape(2, 1), np.float32)),
        ],
        axis=1,
    ).astype(np.float32)

    common = {
        "cbh": np.ascontiguousarray(cbh_pack),
        "cbr": np.ascontiguousarray(cbr_pack),
        "cf": np.ascontiguousarray(cf_pack),
    }

    xg = xp[:, idx, :]  # [B, STEPS, F]
    zsc = (gscale[None, :] * sq[:, None]).copy()  # [STEPS, H]
    # step 0: h=0 so the sigmoid factor is the constant sg0; fold into zs
    sg0 = _sigmoid(np.maximum(gb1, 0.0) @ gw2 + gb2)  # [H]
    zsc[0] *= sg0

    in_maps = []
    for c in range(NCORES):
        rows = slice(c * BC, (c + 1) * BC)
        # (stream, half, b', k, f) -> (k, half, f, stream, b')
        xt = np.ascontiguousarray(
            xg[rows]
            .reshape(2, 2, HB, STEPS, FX)
            .transpose(3, 1, 4, 0, 2)
            .reshape(STEPS, 128, 2, HB)
            .astype(NPBF)
        )
        zc = dw[:, rows, :] * zsc[:, None, :]  # [STEPS, BC, H]
        zst = np.ascontiguousarray(
            zc.reshape(STEPS, 2, 2, HB, H)
            .transpose(0, 2, 4, 1, 3)
            .reshape(STEPS, 128, 2, HB)
            .astype(NPBF)
        )
        m = dict(common)
        m["xt"] = xt
        m["zst"] = zst
        in_maps.append(m)
    return in_maps


def kernel(**inputs):
    nc = _build()
    in_maps = _prep_in_maps(inputs)
    run_kwargs = dict(_CACHE.get("run_kwargs", {}))
    res = run_bass_kernel_spmd(nc, in_maps, list(range(NCORES)), **run_kwargs)
    _CACHE["last_results"] = res
    mus, lss = [], []
    for c in range(NCORES):
        o = res.results[c]["out"]  # [(mu_h0,ls_h0,mu_h1,ls_h1), stream, b]
        mus.append(np.concatenate([o[0, 0], o[2, 0], o[0, 1], o[2, 1]]))
        lss.append(np.concatenate([o[1, 0], o[3, 0], o[1, 1], o[3, 1]]))
    mu = np.concatenate(mus)
    ls = np.concatenate(lss)
    return mu, ls
